# revision 47
# baseline (speedup 1.0000x reference)
"""Trainium2 Bass/Tile kernel for nn_FB_FMM (sparse_attention).

Computation (per batch element b, with N = H*W = 4096 tokens, C=256, D=32):
  1. Self-attention:  sa_out = attn(conv(x,sa_wq), conv(x,sa_wk), conv(x,sa_wv))
     x' = sa_gamma * sa_out + x
  2. Masked cross-attention (FB_FMM):
     ff = mask * x'; fb = (1-mask) * x'
     sw_bg = attn(conv(ff,wq), conv(fb,wk), conv(fb,wv))
     out = x' + gamma * ff * (std(sw_bg)/std(ff))    [per-channel std, ddof=1]

Sharding: 8 cores = 2 batch groups x 4-way query-row sharding (1024 rows each).
Each core computes its row-chunk of both attention layers; K/V sides are
computed redundantly per core. Cross-core communication inside the kernel:
  - AllGather of fb = (1-mask)x' chunks within each 4-core batch group
    (layer-2 K/V convs need full fb; gathering fb instead of x' removes all
    post-gather mask arithmetic), split into two 512-row phases so phase 0
    overlaps the second layer-1 attention i-chunk.
  - AllGather (+ local 3-add sum) of per-channel [sum, sumsq] stats for the
    FMM std ratio -- cheaper than a CC AllReduce for 4KB.

Layouts: feature maps are channel-major (C on partitions). Scores are computed
transposed (S^T: keys j on partitions, queries i free; logits are small so exp
needs no max-subtraction pass). The AV matmul keeps V^T slices stationary
with E^T moving, producing O in natural (c x i) layout; the softmax
denominator comes from one extra M=1 ones-matmul per tile, and the
reciprocal row (reciprocal_approx_fast, ~18 bits) is broadcast across
partitions with a K=1 ones matmul. Layer-1 K/V convs are emitted interleaved
with the i-chunk-0 attention tiles so the PE starts as soon as the first
input chunk lands instead of after the full 2MB x DMA.
"""

import numpy as np

P = 128
B, C, HH, WW = 2, 256, 64, 64
N = HH * WW            # 4096 tokens
D = 32                 # q/k channels
NCORES = 8
RSH = 4                # row shards per batch group
R = N // RSH           # 1024 query rows per core
NT = N // P            # 32 key tiles
IC = 512               # query i-chunk (one PSUM bank of fp32)
EPS = 1e-5
F32 = np.float32

_CACHE = {}


def _build_bass():
    """Build the Bass/Tile program (single SPMD NEFF for all 8 cores)."""
    import concourse.bass as bass
    from concourse import bacc, mybir, tile

    f32 = mybir.dt.float32
    f32r = mybir.dt.float32r
    bf16 = mybir.dt.bfloat16
    AX = mybir.AxisListType
    OP = mybir.AluOpType
    AF = mybir.ActivationFunctionType

    nc = bacc.Bacc(
        "TRN2", target_bir_lowering=False, debug=False, num_devices=NCORES
    )
    bf16d = mybir.dt.bfloat16

    # ---------------- I/O ----------------
    # all inputs are HOST-PACKED to [128, X] with the two channel halves
    # pre-interleaved -- contiguous DMAs run ~3-4x faster than
    # transposed-AP ones
    xf_d = nc.dram_tensor("xf", [P, 2 * N], bf16d, kind="ExternalInput")
    xc_d = nc.dram_tensor("xc", [P, 2 * R], f32, kind="ExternalInput")
    xcb_d = nc.dram_tensor("xcb", [P, 2 * R], bf16d, kind="ExternalInput")
    mcrow_d = nc.dram_tensor("mcrow", [1, R], f32, kind="ExternalInput")
    # packed weights: f32r pack = wq2 only (its moving side ff is f32r;
    # wq1 rides in the bf16 pack since x-local is bf16), bf16 pack =
    # wq1 | wk1 | wk2 | wv1 | wv2 -- 2 DMAs instead of 12
    wpr_d = nc.dram_tensor("wpackr", [P, 2 * D], f32r, kind="ExternalInput")
    wpb_d = nc.dram_tensor(
        "wpackb", [P, 2 * (3 * D + 2 * C)], bf16d, kind="ExternalInput"
    )
    # consts columns: 0 sa_gamma, 1 gamma, 2/3 sa_gamma*sa_bv halves,
    # 6 sa_bq, 7 sa_bk, 8 bq, 9 bk (cols 6-9 live on partitions 0..31)
    consts_d = nc.dram_tensor("consts", [P, 10], f32, kind="ExternalInput")
    out_d = nc.dram_tensor("outc", [C, R], f32, kind="ExternalOutput")

    groups = [[0, 1, 2, 3], [4, 5, 6, 7]]

    with tile.TileContext(nc) as tc:
        from contextlib import ExitStack

        ctx = ExitStack()
        with ctx:
            big = ctx.enter_context(tc.tile_pool(name="big", bufs=1))
            epool = ctx.enter_context(tc.tile_pool(name="epool", bufs=7))
            dspool = ctx.enter_context(tc.tile_pool(name="dspool", bufs=3))
            onpool = ctx.enter_context(tc.tile_pool(name="onpool", bufs=3))
            sqpool = ctx.enter_context(tc.tile_pool(name="sqpool", bufs=2))
            rcpool = ctx.enter_context(tc.tile_pool(name="rcpool", bufs=2))
            finpool = ctx.enter_context(tc.tile_pool(name="finpool", bufs=2))
            l2pool = ctx.enter_context(tc.tile_pool(name="l2pool", bufs=2))
            misc = ctx.enter_context(tc.tile_pool(name="misc", bufs=1))
            # PSUM: accs (3 rotating) + S^T (2) + convs (2) + den/rrep (1)
            psO = ctx.enter_context(
                tc.tile_pool(name="psO", bufs=3, space="PSUM")
            )
            psS = ctx.enter_context(
                tc.tile_pool(name="psS", bufs=2, space="PSUM")
            )
            psC = ctx.enter_context(
                tc.tile_pool(name="psC", bufs=2, space="PSUM")
            )
            psD = ctx.enter_context(
                tc.tile_pool(name="psD", bufs=1, space="PSUM")
            )
            dram = ctx.enter_context(
                tc.tile_pool(name="dram", bufs=1, space="DRAM")
            )

            # ------------- persistent SBUF tiles -------------
            xc_sb = big.tile([P, 2, R], f32, tag="xc", name="xc_sb")
            xcb_sb = big.tile([P, 2, R], bf16, tag="xcb", name="xcb_sb")
            maskc_sb = big.tile([P, R], f32, tag="maskc", name="maskc_sb")
            xp_sb = big.tile([P, 2, R], f32, tag="xp", name="xp_sb")
            ff_sb = big.tile([P, 2, R], f32r, tag="ff", name="ff_sb")
            fb16_sb = big.tile([P, 2, R], bf16, tag="fb16", name="fb16_sb")
            wpr_sb = big.tile([P, 2, D], f32r, tag="wpr", name="wpr_sb")
            wpb_sb = big.tile(
                [P, 2, 3 * D + 2 * C], bf16, tag="wpb", name="wpb_sb"
            )
            consts_sb = big.tile([P, 10], f32, tag="consts", name="consts_sb")
            # ones column (bf16) for the denominator matmul; ones row (f32r)
            # for the K=1 reciprocal-replication matmul
            onesc_sb = big.tile([P, 1], bf16, tag="onesc", name="onesc_sb")
            onesr_sb = big.tile([1, P], f32r, tag="onesr", name="onesr_sb")
            stats_sb = misc.tile([P, 8], f32, tag="stats", name="stats_sb")
            # per-(ct, ich) stat accumulator columns: 4 tensors x [ct, ich]
            sacc_sb = misc.tile([P, 16], f32, tag="sacc", name="sacc_sb")

            # input DMAs: dispatch is ~0.6us per dma_start on one engine's
            # queue, so spread across 4 engine queues and merge the C-half
            # pairs into single transposed-AP transfers.
            # xcb (bf16 local x, feeds the Q1 conv) goes first -- it is the
            # startup critical path; the f32 x copy for the residual isn't
            # needed until epilogue1 so it loads last.
            # gpsimd-initiated DMAs move ~8x slower than the SP/ACT HWDGE
            # queues -- keep every bulk transfer on sync/scalar, gpsimd only
            # gets the tiny wq2 pack
            nc.sync.dma_start(out=xcb_sb[:], in_=xcb_d[:])
            nc.sync.dma_start(out=wpb_sb[:], in_=wpb_d[:])
            nc.sync.dma_start(out=consts_sb[:], in_=consts_d[:])
            nc.gpsimd.dma_start(out=wpr_sb[:], in_=wpr_d[:])
            nc.sync.dma_start(
                out=maskc_sb[:], in_=mcrow_d[0, :].partition_broadcast(P)
            )
            # x-full arrives (ct-interleaved per chunk on the host side) in
            # 4 chunks split across queues; layer-1 convs chase the chunks
            xf_sb = big.tile([P, 2, N], bf16, tag="xbig", name="xf_sb")
            NCH = 4                       # xf DMA chunks
            CW = N // NCH                 # 1024 columns per chunk
            for jc in range(NCH):
                js = slice(jc * CW, (jc + 1) * CW)
                eng = (nc.scalar, nc.sync)[jc % 2]
                eng.dma_start(
                    out=xf_sb[:, :, js],
                    in_=xf_d[:, 2 * jc * CW : 2 * (jc + 1) * CW].rearrange(
                        "p (k n) -> p k n", k=2
                    ),
                )
            nc.scalar.dma_start(out=xc_sb[:], in_=xc_d[:])
            nc.vector.memset(onesc_sb[:], 1.0)
            nc.vector.memset(onesr_sb[:].bitcast(f32), 1.0)

            def conv_qk(wT_sb, bias_col, src_of, cols, out_sb):
                """out[:, cols] (D x 512-chunks) = wT.T @ src + bias."""
                for jc in range(cols.start // IC, cols.stop // IC):
                    js = slice(jc * IC, (jc + 1) * IC)
                    ps = psC.tile([D, IC], f32, tag="c", name="qk_ps")
                    nc.tensor.matmul(
                        ps[:], wT_sb[:, 0, :], src_of(0, js),
                        start=True, stop=False,
                    )
                    nc.tensor.matmul(
                        ps[:], wT_sb[:, 1, :], src_of(1, js),
                        start=False, stop=True,
                    )
                    nc.vector.tensor_scalar_add(
                        out_sb[:, js], ps[:],
                        consts_sb[0:D, bias_col : bias_col + 1],
                    )

            def conv_vT(wvT_sb, src_of, v_sb, t):
                """v_sb[:, t, :] = (src^T @ wvT) for key tile t (j on
                partitions, channels free)."""
                ts_ = slice(t * P, (t + 1) * P)
                ps = psC.tile([P, C], f32, tag="c", name="v_ps")
                nc.tensor.matmul(
                    ps[:], src_of(0, ts_), wvT_sb[:, 0, :],
                    start=True, stop=False,
                )
                nc.tensor.matmul(
                    ps[:], src_of(1, ts_), wvT_sb[:, 1, :],
                    start=False, stop=True,
                )
                nc.vector.tensor_copy(v_sb[:, t, :], ps[:])

            class AttnChunk:
                """One query i-chunk of row-sharded attention, with tile
                emission split into arbitrary sub-sequences so conv work can
                be interleaved.  S^T = K-tile^T Q (j on partitions),
                E = exp(S^T), O accumulates with V^T stationary and E moving;
                denominator via an M=1 ones-matmul per tile."""

                GD = 4  # tiles per denominator group

                def __init__(self, q_sb, k_sb, v_sb, ich, order):
                    self.q_sb, self.k_sb, self.v_sb = q_sb, k_sb, v_sb
                    self.is_ = slice(ich * IC, (ich + 1) * IC)
                    self.order = order
                    self.pos = 0          # next order index to AV
                    self.ahead = 0        # next order index to S/exp
                    self.es = {}
                    self.group = []       # e-tiles awaiting the den group
                    self.gidx = 0
                    self.accs = [
                        psO.tile([P, IC], f32, tag="o", name="acc")
                        for _ in range(2)
                    ]
                    self.den = psD.tile([1, IC], f32, tag="d", name="den")

                def _s_exp(self):
                    t = self.order[self.ahead]
                    self.ahead += 1
                    sps = psS.tile([P, IC], f32, tag="s", name="s_ps")
                    nc.tensor.matmul(
                        sps[:],
                        self.k_sb[:, t * P : (t + 1) * P],
                        self.q_sb[:, self.is_],
                        start=True, stop=True,
                    )
                    e_sb = epool.tile([P, IC], bf16, tag="e", name="e_sb")
                    nc.scalar.activation(e_sb[:], sps[:], AF.Exp)
                    self.es[t] = e_sb

                def emit(self, n):
                    """Emit the next n tiles' worth of S/exp/AV; every GD
                    tiles the e-tiles are tree-summed on the DVE (bf16, 2x
                    mode) and a single ones-matmul accumulates the softmax
                    denominator -- 1 PE matmul + ldweights per GD tiles
                    instead of per tile."""
                    LOOKAHEAD = 1
                    for _ in range(n):
                        while (
                            self.ahead < len(self.order)
                            and self.ahead <= self.pos + LOOKAHEAD
                        ):
                            self._s_exp()
                        t = self.order[self.pos]
                        first = self.pos == 0
                        last = self.pos == len(self.order) - 1
                        self.pos += 1
                        e_sb = self.es.pop(t)
                        for ct in range(2):
                            nc.tensor.matmul(
                                self.accs[ct][:],
                                self.v_sb[:, t, ct * P : (ct + 1) * P],
                                e_sb[:],
                                start=first, stop=last,
                            )
                        self.group.append(e_sb)
                        if len(self.group) == self.GD:
                            g0, g1, g2, g3 = self.group
                            self.group = []
                            e01 = dspool.tile([P, IC], bf16, tag="ds",
                                              name="e01")
                            nc.vector.tensor_add(e01[:], g0[:], g1[:])
                            e23 = dspool.tile([P, IC], bf16, tag="ds",
                                              name="e23")
                            nc.vector.tensor_add(e23[:], g2[:], g3[:])
                            esum = dspool.tile([P, IC], bf16, tag="ds",
                                               name="esum")
                            nc.vector.tensor_add(esum[:], e01[:], e23[:])
                            ng = len(self.order) // self.GD
                            nc.tensor.matmul(
                                self.den[:], onesc_sb[:], esum[:],
                                start=self.gidx == 0,
                                stop=self.gidx == ng - 1,
                            )
                            self.gidx += 1

                def rrep(self):
                    """Reciprocal of the denominator row, replicated to all
                    partitions via a K=1 ones matmul (f32r single-pass).
                    reciprocal_approx_fast (~18 bits) replaces the 5x slower
                    exact DVE reciprocal; the f32r copy rounds for the PE."""
                    rrow = rcpool.tile([1, IC], f32, tag="rc0", name="rw")
                    nc.vector.reciprocal_approx_fast(
                        out=rrow[:], in_=self.den[:]
                    )
                    rrow_r = rcpool.tile([1, IC], f32r, tag="rc", name="rr")
                    nc.vector.tensor_copy(rrow_r[:], rrow[:])
                    rrep_ps = psD.tile([P, IC], f32, tag="d", name="rrep_ps")
                    nc.tensor.matmul(
                        rrep_ps[:], onesr_sb[:], rrow_r[:],
                        start=True, stop=True,
                    )
                    rr = onpool.tile([P, IC], f32, tag="rr", name="rrep")
                    nc.scalar.copy(rr[:], rrep_ps[:])
                    return rr

            # ================= Layer 1: self-attention =================
            q1_sb = big.tile([D, R], bf16, tag="q", name="q1_sb")
            k1_sb = big.tile([D, N], bf16, tag="k", name="k1_sb")
            v1_sb = big.tile([P, NT, C], bf16, tag="v", name="v1_sb")

            wq2 = wpr_sb
            wq1 = wpb_sb[:, :, 0:D]
            wk1 = wpb_sb[:, :, D : 2 * D]
            wk2 = wpb_sb[:, :, 2 * D : 3 * D]
            wv1 = wpb_sb[:, :, 3 * D : 3 * D + C]
            wv2 = wpb_sb[:, :, 3 * D + C : 3 * D + 2 * C]

            conv_qk(wq1, 6, lambda k, js: xcb_sb[:, k, js], slice(0, R),
                    q1_sb)

            def l1_conv_chunk(c):
                js = slice(c * CW, (c + 1) * CW)
                conv_qk(wk1, 7, lambda k, js_: xf_sb[:, k, js_], js, k1_sb)
                for t in range(8 * c, 8 * c + 8):
                    conv_vT(wv1, lambda k, ts_: xf_sb[:, k, ts_], v1_sb, t)

            # i-chunk 0 attention trails the conv chunks by one chunk
            at0 = AttnChunk(q1_sb, k1_sb, v1_sb, 0, list(range(NT)))
            l1_conv_chunk(0)
            for c in range(1, NCH):
                l1_conv_chunk(c)
                at0.emit(8)
            at0.emit(8)

            # gather buffers carry the LOCAL layer-2 conv outputs (K2 rows
            # 0..D, V2^T rows D..D+2P) so the remote 3/4 of the K2/V2 convs
            # never run here -- 1 gather per i-chunk, nothing to conv after.
            AGR = D + 2 * P
            ag_ins, ag_outs = [], []
            for h in range(2):
                ag_ins.append(dram.tile(
                    [AGR, IC], bf16, tag=f"ag_in{h}", name=f"ag_in{h}"
                ))
                ag_outs.append(dram.tile(
                    [RSH, AGR, IC], bf16, tag=f"ag_out{h}", name=f"ag_out{h}"
                ))
            k2_sb = big.tile([D, N], bf16, tag="k2", name="k2_sb")
            q2_sb = big.tile([D, R], bf16, tag="q2", name="q2_sb")

            def epilogue1(ich, at):
                """x' = sa_gamma*(O/den) + sa_gamma*bv + x, then ff = m*x'
                (f32r, with Σff accumulated), fb16 = x' - ff, the local
                K2/V2 conv chunk + its AllGather, Σff², and the q2 conv."""
                rr = at.rrep()
                io = slice(ich * IC, (ich + 1) * IC)
                for ct in range(2):
                    nc.vector.scalar_tensor_tensor(
                        xp_sb[:, ct, io], at.accs[ct][:],
                        consts_sb[:, 0:1], rr[:],
                        op0=OP.mult, op1=OP.mult,
                    )
                    nc.vector.scalar_tensor_tensor(
                        xp_sb[:, ct, io], xp_sb[:, ct, io],
                        consts_sb[:, 2 + ct : 3 + ct],
                        xc_sb[:, ct, io],
                        op0=OP.add, op1=OP.add,
                    )
                    nc.vector.scalar_tensor_tensor(
                        ff_sb[:, ct, io], xp_sb[:, ct, io],
                        1.0, maskc_sb[:, io],
                        op0=OP.mult, op1=OP.mult,
                        accum_out=sacc_sb[:, 2 * ct + ich : 2 * ct + ich + 1],
                    )
                    nc.vector.tensor_sub(
                        fb16_sb[:, ct, io], xp_sb[:, ct, io],
                        ff_sb[:, ct, io].bitcast(f32),
                    )
                # local K2 conv chunk
                k2l = l2pool.tile([D, IC], bf16, tag="k2l", name="k2l")
                ps = psC.tile([D, IC], f32, tag="c", name="k2l_ps")
                nc.tensor.matmul(
                    ps[:], wk2[:, 0, :], fb16_sb[:, 0, io],
                    start=True, stop=False,
                )
                nc.tensor.matmul(
                    ps[:], wk2[:, 1, :], fb16_sb[:, 1, io],
                    start=False, stop=True,
                )
                # PSUM->SBUF copies on the Scalar engine: the DVE is backed
                # up with epilogue STTs here and would delay the gather
                nc.scalar.add(k2l[:], ps[:], consts_sb[0:D, 9:10])
                # local V2^T conv tiles
                v2l = l2pool.tile([P, 4, C], bf16, tag="v2l", name="v2l")
                for tsub in range(4):
                    cs = slice(io.start + tsub * P, io.start + (tsub + 1) * P)
                    psv = psC.tile([P, C], f32, tag="c", name="v2l_ps")
                    nc.tensor.matmul(
                        psv[:], fb16_sb[:, 0, cs], wv2[:, 0, :],
                        start=True, stop=False,
                    )
                    nc.tensor.matmul(
                        psv[:], fb16_sb[:, 1, cs], wv2[:, 1, :],
                        start=False, stop=True,
                    )
                    nc.scalar.copy(v2l[:, tsub, :], psv[:])
                nc.sync.dma_start(out=ag_ins[ich][0:D, :], in_=k2l[:])
                nc.scalar.dma_start(out=ag_ins[ich][D:, :], in_=v2l[:])
                nc.gpsimd.collective_compute(
                    "AllGather",
                    OP.bypass,
                    replica_groups=groups,
                    ins=[ag_ins[ich][:].opt()],
                    outs=[ag_outs[ich][:].opt()],
                )
                # Σff² (scratch output, accumulator is the point)
                for ct in range(2):
                    sq = sqpool.tile([P, IC], f32, tag="sq", name="ffsq")
                    nc.vector.scalar_tensor_tensor(
                        sq[:], ff_sb[:, ct, io].bitcast(f32),
                        1.0, ff_sb[:, ct, io].bitcast(f32),
                        op0=OP.mult, op1=OP.mult,
                        accum_out=sacc_sb[:, 4 + 2 * ct + ich
                                          : 5 + 2 * ct + ich],
                    )
                conv_qk(wq2, 8, lambda k, js: ff_sb[:, k, js], io, q2_sb)

            epilogue1(0, at0)
            del at0

            at1 = AttnChunk(q1_sb, k1_sb, v1_sb, 1, list(range(NT)))
            at1.emit(NT)
            epilogue1(1, at1)
            del at1

            # assemble ff stats: stats cols 0-1 = S1f (ct0, ct1),
            # cols 4-5 = S2f -- S1/S2 grouped 4-wide for the var chain
            for c in range(4):
                dst = (0, 1, 4, 5)[c]
                nc.vector.tensor_add(
                    stats_sb[:, dst : dst + 1],
                    sacc_sb[:, 2 * c : 2 * c + 1],
                    sacc_sb[:, 2 * c + 1 : 2 * c + 2],
                )

            # ============== Layer 2: masked cross-attention ==============
            # redistribute the gathered K2/V2 conv outputs: 2 transposed
            # DMAs per phase instead of per-rank copies
            v2_sb = big.tile([P, NT, C], bf16, tag="v2", name="v2_sb")
            for h in range(2):
                eng = (nc.sync, nc.scalar)[h]
                eng.dma_start(
                    out=k2_sb[:].rearrange("d (r q) -> d r q", r=RSH)[
                        :, :, h * IC : (h + 1) * IC
                    ],
                    in_=ag_outs[h][:, 0:D, :].transpose([1, 0, 2]),
                )
                eng.dma_start(
                    out=v2_sb[:].rearrange("p (r t) c -> p r t c", r=RSH)[
                        :, :, 4 * h : 4 * h + 4, :
                    ],
                    in_=ag_outs[h][:, D:, :].rearrange(
                        "r (p j) (x c) -> p r (j x) c", j=2, x=2
                    ),
                )

            ph_tiles = [
                [t for r in range(RSH)
                 for t in range((r * R + h * IC) // P,
                                (r * R + h * IC) // P + 4)]
                for h in range(2)
            ]

            bt0 = AttnChunk(q2_sb, k2_sb, v2_sb, 0,
                            ph_tiles[0] + ph_tiles[1])
            bt0.emit(NT)

            def epilogue2(ich, at):
                """normalized sw_bg chunk; accumulate per-channel sum/sumsq
                into sacc_sb cols 8-15 via the DVE accumulator."""
                rr = at.rrep()
                for ct in range(2):
                    onb = onpool.tile([P, IC], f32, tag="rr", name="on2")
                    nc.vector.scalar_tensor_tensor(
                        onb[:], at.accs[ct][:], 1.0, rr[:],
                        op0=OP.mult, op1=OP.mult,
                        accum_out=sacc_sb[:, 8 + 2 * ct + ich
                                          : 9 + 2 * ct + ich],
                    )
                    sqb = sqpool.tile([P, IC], f32, tag="sq", name="sq2")
                    nc.vector.scalar_tensor_tensor(
                        sqb[:], onb[:], 1.0, onb[:],
                        op0=OP.mult, op1=OP.mult,
                        accum_out=sacc_sb[:, 12 + 2 * ct + ich
                                          : 13 + 2 * ct + ich],
                    )

            epilogue2(0, bt0)
            del bt0
            bt1 = AttnChunk(q2_sb, k2_sb, v2_sb, 1, list(range(NT)))
            bt1.emit(NT)
            # preload the Sqrt ACT table while the last AV matmuls drain so
            # the tail's real sqrt skips the 1.3us table switch
            sqwarm = misc.tile([P, 1], f32, tag="sqwarm", name="sqwarm")
            nc.scalar.activation(sqwarm[:], consts_sb[:, 0:1], AF.Sqrt)
            epilogue2(1, bt1)
            del bt1

            # assemble sw_bg stats: stats cols 2-3 = S1g, cols 6-7 = S2g
            for c in range(4):
                dst = (2, 3, 6, 7)[c]
                nc.vector.tensor_add(
                    stats_sb[:, dst : dst + 1],
                    sacc_sb[:, 8 + 2 * c : 9 + 2 * c],
                    sacc_sb[:, 9 + 2 * c : 10 + 2 * c],
                )

            # ============ stats AllGather + local sum + FMM ============
            sg_in = dram.tile([P, 8], f32, tag="sg_in", name="sg_in")
            sg_out = dram.tile([RSH, P, 8], f32, tag="sg_out", name="sg_out")
            nc.sync.dma_start(out=sg_in[:], in_=stats_sb[:])
            nc.gpsimd.collective_compute(
                "AllGather",
                OP.bypass,
                replica_groups=groups,
                ins=[sg_in[:].opt()],
                outs=[sg_out[:].opt()],
            )
            rst4 = misc.tile([P, RSH, 8], f32, tag="rst4", name="rst4")
            nc.sync.dma_start(
                out=rst4[:], in_=sg_out[:].transpose([1, 0, 2])
            )
            rst = misc.tile([P, 8], f32, tag="rst", name="rst")
            nc.vector.tensor_add(rst[:], rst4[:, 0, :], rst4[:, 1, :])
            rstb = misc.tile([P, 8], f32, tag="rstb", name="rstb")
            nc.vector.tensor_add(rstb[:], rst4[:, 2, :], rst4[:, 3, :])
            nc.vector.tensor_add(rst[:], rst[:], rstb[:])

            # var = (S2 - S1^2/N)/(N-1) + EPS, all four (f/g x ct) at once
            var4 = misc.tile([P, 4], f32, tag="var4", name="var4")
            ratio = misc.tile([P, 2], f32, tag="ratio", name="ratio")
            nc.vector.tensor_mul(var4[:], rst[:, 0:4], rst[:, 0:4])
            nc.vector.tensor_scalar(
                var4[:], var4[:], -1.0 / N, None, op0=OP.mult
            )
            nc.vector.tensor_add(var4[:], var4[:], rst[:, 4:8])
            nc.vector.tensor_scalar(
                var4[:], var4[:], 1.0 / (N - 1), EPS, op0=OP.mult, op1=OP.add
            )
            varfi = misc.tile([P, 2], f32, tag="varfi", name="varfi")
            nc.vector.reciprocal(varfi[:], var4[:, 0:2])
            nc.vector.tensor_mul(varfi[:], varfi[:], var4[:, 2:4])
            nc.scalar.activation(ratio[:], varfi[:], AF.Sqrt)
            # fold in gamma
            nc.vector.tensor_scalar_mul(ratio[:], ratio[:], consts_sb[:, 1:2])

            # out = x' + (gamma * std_bg/std_f) * ff, in 4 chunks with the
            # store DMAs dispatched on 4 different engine queues
            outq = (nc.sync, nc.scalar, nc.sync, nc.scalar)
            for i, (ct, ih) in enumerate(
                ((0, 0), (1, 0), (0, 1), (1, 1))
            ):
                io = slice(ih * IC, (ih + 1) * IC)
                fin = finpool.tile([P, IC], f32, tag="fin", name="fin")
                nc.vector.scalar_tensor_tensor(
                    fin[:], ff_sb[:, ct, io].bitcast(f32),
                    ratio[:, ct : ct + 1], xp_sb[:, ct, io],
                    op0=OP.mult, op1=OP.add,
                )
                outq[i].dma_start(
                    out=out_d[ct * P : (ct + 1) * P, io], in_=fin[:]
                )

    nc.compile()
    return nc


def _prep_inputs(x, mask, sa_wq, sa_bq, sa_wk, sa_bk, sa_wv, sa_bv, sa_gamma,
                 wq, bq, wk, bk, wv, bv, gamma):
    """Build the per-core input maps (host-side sharding + weight layout)."""
    x = np.ascontiguousarray(x, dtype=F32)
    mask = np.ascontiguousarray(mask, dtype=F32)

    import ml_dtypes

    BF16 = ml_dtypes.bfloat16
    def chalf(a):
        """[C, X] -> [128, 2*X] with the two channel halves interleaved
        per partition (so device DMAs are contiguous)."""
        X = a.shape[1]
        return np.ascontiguousarray(
            a.reshape(2, P, X).transpose(1, 0, 2).reshape(P, 2 * X)
        )

    # packed weights: f32r pack = wq2; bf16 pack = wq1|wk1|wk2|wv1|wv2
    wpackr = chalf(np.ascontiguousarray(wq.T, dtype=F32))
    wpackb = chalf(
        np.concatenate(
            [sa_wq.T, sa_wk.T, wk.T, sa_wv.T, wv.T], axis=1
        ).astype(BF16)
    )

    consts = np.zeros((P, 10), dtype=F32)
    consts[:, 0] = sa_gamma[0]
    consts[:, 1] = gamma[0]
    sgb = (sa_gamma[0] * sa_bv).astype(F32)
    consts[:, 2] = sgb[0:P]
    consts[:, 3] = sgb[P:C]
    consts[0:D, 6] = sa_bq
    consts[0:D, 7] = sa_bk
    consts[0:D, 8] = bq
    consts[0:D, 9] = bk

    in_maps = []
    for g in range(NCORES):
        b, r = g // RSH, g % RSH
        xb = np.ascontiguousarray(x[b].reshape(C, N))
        mb = np.ascontiguousarray(mask[b].reshape(1, N))
        xloc = xb[:, r * R : (r + 1) * R]
        # xf: [128, NCH, 2, CW] host layout -> per-chunk contiguous DMAs
        NCH, CW = 4, N // 4
        xfp = np.ascontiguousarray(
            xb.astype(BF16).reshape(2, P, NCH, CW)
            .transpose(1, 2, 0, 3).reshape(P, 2 * N)
        )
        in_maps.append({
            "xf": xfp,
            "xc": chalf(xloc),
            "xcb": chalf(xloc.astype(BF16)),
            "mcrow": np.ascontiguousarray(mb[:, r * R : (r + 1) * R]),
            "wpackr": wpackr, "wpackb": wpackb,
            "consts": consts,
        })
    return in_maps


def kernel(**inputs):
    from concourse import bass_utils

    if "nc" not in _CACHE:
        _CACHE["nc"] = _build_bass()
    nc = _CACHE["nc"]

    in_maps = _prep_inputs(**inputs)
    res = bass_utils.run_bass_kernel_spmd(
        nc, in_maps, core_ids=list(range(NCORES))
    )
    _CACHE["last_results"] = res

    out = np.empty((B, C, N), dtype=F32)
    for g in range(NCORES):
        b, r = g // RSH, g % RSH
        out[b, :, r * R : (r + 1) * R] = res.results[g]["outc"]
    return out.reshape(B, C, HH, WW)


# revision 51
# speedup vs baseline: 1.0978x; 1.0978x over previous
"""Trainium2 Bass/Tile kernel for nn_FB_FMM (sparse_attention).

Computation (per batch element b, with N = H*W = 4096 tokens, C=256, D=32):
  1. Self-attention:  sa_out = attn(conv(x,sa_wq), conv(x,sa_wk), conv(x,sa_wv))
     x' = sa_gamma * sa_out + x
  2. Masked cross-attention (FB_FMM):
     ff = mask * x'; fb = (1-mask) * x'
     sw_bg = attn(conv(ff,wq), conv(fb,wk), conv(fb,wv))
     out = x' + gamma * ff * (std(sw_bg)/std(ff))    [per-channel std, ddof=1]

Sharding: 8 cores = 2 batch groups x 4-way query-row sharding (1024 rows each).
Each core computes its row-chunk of both attention layers; K/V sides are
computed redundantly per core. Cross-core communication inside the kernel:
  - AllGather of fb = (1-mask)x' chunks within each 4-core batch group
    (layer-2 K/V convs need full fb; gathering fb instead of x' removes all
    post-gather mask arithmetic), split into two 512-row phases so phase 0
    overlaps the second layer-1 attention i-chunk.
  - AllGather (+ local 3-add sum) of per-channel [sum, sumsq] stats for the
    FMM std ratio -- cheaper than a CC AllReduce for 4KB.

Layouts: feature maps are channel-major (C on partitions). Scores are computed
transposed (S^T: keys j on partitions, queries i free; logits are small so exp
needs no max-subtraction pass). The AV matmul keeps V^T slices stationary
with E^T moving, producing O in natural (c x i) layout; the softmax
denominator comes from one extra M=1 ones-matmul per tile, and the
reciprocal row (reciprocal_approx_fast, ~18 bits) is broadcast across
partitions with a K=1 ones matmul. Layer-1 K/V convs are emitted interleaved
with the i-chunk-0 attention tiles so the PE starts as soon as the first
input chunk lands instead of after the full 2MB x DMA.
"""

import numpy as np

P = 128
B, C, HH, WW = 2, 256, 64, 64
N = HH * WW            # 4096 tokens
D = 32                 # q/k channels
NCORES = 8
RSH = 4                # row shards per batch group
R = N // RSH           # 1024 query rows per core
NT = N // P            # 32 key tiles
IC = 512               # query i-chunk (one PSUM bank of fp32)
EPS = 1e-5
F32 = np.float32

_CACHE = {}


def _build_bass():
    """Build the Bass/Tile program (single SPMD NEFF for all 8 cores)."""
    import concourse.bass as bass
    from concourse import bacc, mybir, tile

    f32 = mybir.dt.float32
    f32r = mybir.dt.float32r
    bf16 = mybir.dt.bfloat16
    AX = mybir.AxisListType
    OP = mybir.AluOpType
    AF = mybir.ActivationFunctionType

    nc = bacc.Bacc(
        "TRN2", target_bir_lowering=False, debug=False, num_devices=NCORES
    )
    bf16d = mybir.dt.bfloat16

    # ---------------- I/O ----------------
    # all inputs are HOST-PACKED to [128, X] with the two channel halves
    # pre-interleaved -- contiguous DMAs run ~3-4x faster than
    # transposed-AP ones
    xf_d = nc.dram_tensor("xf", [P, 2 * N], bf16d, kind="ExternalInput")
    xc_d = nc.dram_tensor("xc", [P, 2 * R], f32, kind="ExternalInput")
    xcb_d = nc.dram_tensor("xcb", [P, 2 * R], bf16d, kind="ExternalInput")
    mcrow_d = nc.dram_tensor("mcrow", [1, R], f32, kind="ExternalInput")
    # packed weights: f32r pack = wq2 only (its moving side ff is f32r;
    # wq1 rides in the bf16 pack since x-local is bf16), bf16 pack =
    # wq1 | wk1 | wk2 | wv1 | wv2 -- 2 DMAs instead of 12
    wpr_d = nc.dram_tensor("wpackr", [P, 2 * D], f32r, kind="ExternalInput")
    wpb_d = nc.dram_tensor(
        "wpackb", [P, 2 * (3 * D + 2 * C)], bf16d, kind="ExternalInput"
    )
    # consts columns: 0 sa_gamma, 1 gamma, 2/3 sa_gamma*sa_bv halves,
    # 6 sa_bq, 7 sa_bk, 8 bq, 9 bk (cols 6-9 live on partitions 0..31)
    consts_d = nc.dram_tensor("consts", [P, 10], f32, kind="ExternalInput")
    out_d = nc.dram_tensor("outc", [C, R], f32, kind="ExternalOutput")

    groups = [[0, 1, 2, 3], [4, 5, 6, 7]]

    with tile.TileContext(nc) as tc:
        from contextlib import ExitStack

        ctx = ExitStack()
        with ctx:
            big = ctx.enter_context(tc.tile_pool(name="big", bufs=1))
            epool = ctx.enter_context(tc.tile_pool(name="epool", bufs=7))
            dspool = ctx.enter_context(tc.tile_pool(name="dspool", bufs=3))
            onpool = ctx.enter_context(tc.tile_pool(name="onpool", bufs=3))
            sqpool = ctx.enter_context(tc.tile_pool(name="sqpool", bufs=2))
            rcpool = ctx.enter_context(tc.tile_pool(name="rcpool", bufs=2))
            finpool = ctx.enter_context(tc.tile_pool(name="finpool", bufs=2))
            l2pool = ctx.enter_context(tc.tile_pool(name="l2pool", bufs=2))
            misc = ctx.enter_context(tc.tile_pool(name="misc", bufs=1))
            # PSUM: accs (3 rotating) + S^T (2) + convs (2) + den/rrep (1)
            psO = ctx.enter_context(
                tc.tile_pool(name="psO", bufs=3, space="PSUM")
            )
            psS = ctx.enter_context(
                tc.tile_pool(name="psS", bufs=2, space="PSUM")
            )
            psC = ctx.enter_context(
                tc.tile_pool(name="psC", bufs=2, space="PSUM")
            )
            psD = ctx.enter_context(
                tc.tile_pool(name="psD", bufs=1, space="PSUM")
            )
            dram = ctx.enter_context(
                tc.tile_pool(name="dram", bufs=1, space="DRAM")
            )

            # ------------- persistent SBUF tiles -------------
            xc_sb = big.tile([P, 2, R], f32, tag="xc", name="xc_sb")
            xcb_sb = big.tile([P, 2, R], bf16, tag="xcb", name="xcb_sb")
            maskc_sb = big.tile([P, R], f32, tag="maskc", name="maskc_sb")
            xp_sb = big.tile([P, 2, R], f32, tag="xp", name="xp_sb")
            ff_sb = big.tile([P, 2, R], f32r, tag="ff", name="ff_sb")
            fb16_sb = big.tile([P, 2, R], bf16, tag="fb16", name="fb16_sb")
            wpr_sb = big.tile([P, 2, D], f32r, tag="wpr", name="wpr_sb")
            wpb_sb = big.tile(
                [P, 2, 3 * D + 2 * C], bf16, tag="wpb", name="wpb_sb"
            )
            consts_sb = big.tile([P, 10], f32, tag="consts", name="consts_sb")
            # ones column (bf16) for the denominator matmul; ones row (f32r)
            # for the K=1 reciprocal-replication matmul
            onesc_sb = big.tile([P, 1], bf16, tag="onesc", name="onesc_sb")
            onesr_sb = big.tile([1, P], f32r, tag="onesr", name="onesr_sb")
            stats_sb = misc.tile([P, 8], f32, tag="stats", name="stats_sb")
            # per-(ct, ich) stat accumulator columns: 4 tensors x [ct, ich]
            sacc_sb = misc.tile([P, 16], f32, tag="sacc", name="sacc_sb")

            # input DMAs: dispatch is ~0.6us per dma_start on one engine's
            # queue, so spread across 4 engine queues and merge the C-half
            # pairs into single transposed-AP transfers.
            # xcb (bf16 local x, feeds the Q1 conv) goes first -- it is the
            # startup critical path; the f32 x copy for the residual isn't
            # needed until epilogue1 so it loads last.
            # gpsimd-initiated DMAs move ~8x slower than the SP/ACT HWDGE
            # queues -- keep every bulk transfer on sync/scalar, gpsimd only
            # gets the tiny wq2 pack
            nc.sync.dma_start(out=xcb_sb[:], in_=xcb_d[:])
            nc.sync.dma_start(out=wpb_sb[:], in_=wpb_d[:])
            nc.sync.dma_start(out=consts_sb[:], in_=consts_d[:])
            nc.gpsimd.dma_start(out=wpr_sb[:], in_=wpr_d[:])
            nc.sync.dma_start(
                out=maskc_sb[:], in_=mcrow_d[0, :].partition_broadcast(P)
            )
            # x-full arrives (ct-interleaved per chunk on the host side) in
            # 4 chunks split across queues; layer-1 convs chase the chunks
            xf_sb = big.tile([P, 2, N], bf16, tag="xbig", name="xf_sb")
            NCH = 4                       # xf DMA chunks
            CW = N // NCH                 # 1024 columns per chunk
            for jc in range(NCH):
                js = slice(jc * CW, (jc + 1) * CW)
                eng = (nc.scalar, nc.gpsimd)[jc % 2]
                eng.dma_start(
                    out=xf_sb[:, :, js],
                    in_=xf_d[:, 2 * jc * CW : 2 * (jc + 1) * CW].rearrange(
                        "p (k n) -> p k n", k=2
                    ),
                )
            nc.scalar.dma_start(out=xc_sb[:], in_=xc_d[:])
            nc.vector.memset(onesc_sb[:], 1.0)
            nc.vector.memset(onesr_sb[:].bitcast(f32), 1.0)

            def conv_qk(wT_sb, bias_col, src_of, cols, out_sb):
                """out[:, cols] (D x 512-chunks) = wT.T @ src + bias."""
                for jc in range(cols.start // IC, cols.stop // IC):
                    js = slice(jc * IC, (jc + 1) * IC)
                    ps = psC.tile([D, IC], f32, tag="c", name="qk_ps")
                    nc.tensor.matmul(
                        ps[:], wT_sb[:, 0, :], src_of(0, js),
                        start=True, stop=False,
                    )
                    nc.tensor.matmul(
                        ps[:], wT_sb[:, 1, :], src_of(1, js),
                        start=False, stop=True,
                    )
                    nc.vector.tensor_scalar_add(
                        out_sb[:, js], ps[:],
                        consts_sb[0:D, bias_col : bias_col + 1],
                    )

            def conv_vT(wvT_sb, src_of, v_sb, t):
                """v_sb[:, t, :] = (src^T @ wvT) for key tile t (j on
                partitions, channels free)."""
                ts_ = slice(t * P, (t + 1) * P)
                ps = psC.tile([P, C], f32, tag="c", name="v_ps")
                nc.tensor.matmul(
                    ps[:], src_of(0, ts_), wvT_sb[:, 0, :],
                    start=True, stop=False,
                )
                nc.tensor.matmul(
                    ps[:], src_of(1, ts_), wvT_sb[:, 1, :],
                    start=False, stop=True,
                )
                nc.vector.tensor_copy(v_sb[:, t, :], ps[:])

            class AttnChunk:
                """One query i-chunk of row-sharded attention, with tile
                emission split into arbitrary sub-sequences so conv work can
                be interleaved.  S^T = K-tile^T Q (j on partitions),
                E = exp(S^T), O accumulates with V^T stationary and E moving;
                denominator via an M=1 ones-matmul per tile."""

                GD = 4  # tiles per denominator group

                def __init__(self, q_sb, k_sb, v_sb, ich, order):
                    self.q_sb, self.k_sb, self.v_sb = q_sb, k_sb, v_sb
                    self.is_ = slice(ich * IC, (ich + 1) * IC)
                    self.order = order
                    self.pos = 0          # next order index to AV
                    self.ahead = 0        # next order index to S/exp
                    self.es = {}
                    self.group = []       # e-tiles awaiting the den group
                    self.gidx = 0
                    self.accs = [
                        psO.tile([P, IC], f32, tag="o", name="acc")
                        for _ in range(2)
                    ]
                    self.den = psD.tile([1, IC], f32, tag="d", name="den")

                def _s_exp(self):
                    t = self.order[self.ahead]
                    self.ahead += 1
                    sps = psS.tile([P, IC], f32, tag="s", name="s_ps")
                    nc.tensor.matmul(
                        sps[:],
                        self.k_sb[:, t * P : (t + 1) * P],
                        self.q_sb[:, self.is_],
                        start=True, stop=True,
                    )
                    e_sb = epool.tile([P, IC], bf16, tag="e", name="e_sb")
                    nc.scalar.activation(e_sb[:], sps[:], AF.Exp)
                    self.es[t] = e_sb

                def emit(self, n):
                    """Emit the next n tiles' worth of S/exp/AV; every GD
                    tiles the e-tiles are tree-summed on the DVE (bf16, 2x
                    mode) and a single ones-matmul accumulates the softmax
                    denominator -- 1 PE matmul + ldweights per GD tiles
                    instead of per tile."""
                    LOOKAHEAD = 1
                    for _ in range(n):
                        while (
                            self.ahead < len(self.order)
                            and self.ahead <= self.pos + LOOKAHEAD
                        ):
                            self._s_exp()
                        t = self.order[self.pos]
                        first = self.pos == 0
                        last = self.pos == len(self.order) - 1
                        self.pos += 1
                        e_sb = self.es.pop(t)
                        for ct in range(2):
                            nc.tensor.matmul(
                                self.accs[ct][:],
                                self.v_sb[:, t, ct * P : (ct + 1) * P],
                                e_sb[:],
                                start=first, stop=last,
                            )
                        self.group.append(e_sb)
                        if len(self.group) == self.GD:
                            g0, g1, g2, g3 = self.group
                            self.group = []
                            e01 = dspool.tile([P, IC], bf16, tag="ds",
                                              name="e01")
                            nc.vector.tensor_add(e01[:], g0[:], g1[:])
                            e23 = dspool.tile([P, IC], bf16, tag="ds",
                                              name="e23")
                            nc.vector.tensor_add(e23[:], g2[:], g3[:])
                            esum = dspool.tile([P, IC], bf16, tag="ds",
                                               name="esum")
                            nc.vector.tensor_add(esum[:], e01[:], e23[:])
                            ng = len(self.order) // self.GD
                            nc.tensor.matmul(
                                self.den[:], onesc_sb[:], esum[:],
                                start=self.gidx == 0,
                                stop=self.gidx == ng - 1,
                            )
                            self.gidx += 1

                def rrep(self):
                    """Reciprocal of the denominator row, replicated to all
                    partitions via a K=1 ones matmul (f32r single-pass).
                    reciprocal_approx_fast (~18 bits) replaces the 5x slower
                    exact DVE reciprocal; the f32r copy rounds for the PE."""
                    rrow = rcpool.tile([1, IC], f32, tag="rc0", name="rw")
                    nc.vector.reciprocal_approx_fast(
                        out=rrow[:], in_=self.den[:]
                    )
                    rrow_r = rcpool.tile([1, IC], f32r, tag="rc", name="rr")
                    nc.vector.tensor_copy(rrow_r[:], rrow[:])
                    rrep_ps = psD.tile([P, IC], f32, tag="d", name="rrep_ps")
                    nc.tensor.matmul(
                        rrep_ps[:], onesr_sb[:], rrow_r[:],
                        start=True, stop=True,
                    )
                    rr = onpool.tile([P, IC], f32, tag="rr", name="rrep")
                    nc.scalar.copy(rr[:], rrep_ps[:])
                    return rr

            # ================= Layer 1: self-attention =================
            q1_sb = big.tile([D, R], bf16, tag="q", name="q1_sb")
            k1_sb = big.tile([D, N], bf16, tag="k", name="k1_sb")
            v1_sb = big.tile([P, NT, C], bf16, tag="v", name="v1_sb")

            wq2 = wpr_sb
            wq1 = wpb_sb[:, :, 0:D]
            wk1 = wpb_sb[:, :, D : 2 * D]
            wk2 = wpb_sb[:, :, 2 * D : 3 * D]
            wv1 = wpb_sb[:, :, 3 * D : 3 * D + C]
            wv2 = wpb_sb[:, :, 3 * D + C : 3 * D + 2 * C]

            conv_qk(wq1, 6, lambda k, js: xcb_sb[:, k, js], slice(0, R),
                    q1_sb)

            def l1_conv_chunk(c):
                js = slice(c * CW, (c + 1) * CW)
                conv_qk(wk1, 7, lambda k, js_: xf_sb[:, k, js_], js, k1_sb)
                for t in range(8 * c, 8 * c + 8):
                    conv_vT(wv1, lambda k, ts_: xf_sb[:, k, ts_], v1_sb, t)

            # i-chunk 0 attention trails the conv chunks by one chunk
            at0 = AttnChunk(q1_sb, k1_sb, v1_sb, 0, list(range(NT)))
            l1_conv_chunk(0)
            for c in range(1, NCH):
                l1_conv_chunk(c)
                at0.emit(8)
            at0.emit(8)

            # gather buffers carry the LOCAL layer-2 conv outputs (K2 rows
            # 0..D, V2^T rows D..D+2P) so the remote 3/4 of the K2/V2 convs
            # never run here -- 1 gather per i-chunk, nothing to conv after.
            AGR = D + 2 * P
            ag_ins, ag_outs = [], []
            for h in range(2):
                ag_ins.append(dram.tile(
                    [AGR, IC], bf16, tag=f"ag_in{h}", name=f"ag_in{h}"
                ))
                ag_outs.append(dram.tile(
                    [RSH, AGR, IC], bf16, tag=f"ag_out{h}", name=f"ag_out{h}"
                ))
            k2_sb = big.tile([D, N], bf16, tag="k2", name="k2_sb")
            q2_sb = big.tile([D, R], bf16, tag="q2", name="q2_sb")

            def epilogue1(ich, at):
                """x' = sa_gamma*(O/den) + sa_gamma*bv + x, then ff = m*x'
                (f32r, with Σff accumulated), fb16 = x' - ff, the local
                K2/V2 conv chunk + its AllGather, Σff², and the q2 conv."""
                rr = at.rrep()
                io = slice(ich * IC, (ich + 1) * IC)
                for ct in range(2):
                    nc.vector.scalar_tensor_tensor(
                        xp_sb[:, ct, io], at.accs[ct][:],
                        consts_sb[:, 0:1], rr[:],
                        op0=OP.mult, op1=OP.mult,
                    )
                    nc.vector.scalar_tensor_tensor(
                        xp_sb[:, ct, io], xp_sb[:, ct, io],
                        consts_sb[:, 2 + ct : 3 + ct],
                        xc_sb[:, ct, io],
                        op0=OP.add, op1=OP.add,
                    )
                    nc.vector.scalar_tensor_tensor(
                        ff_sb[:, ct, io], xp_sb[:, ct, io],
                        1.0, maskc_sb[:, io],
                        op0=OP.mult, op1=OP.mult,
                        accum_out=sacc_sb[:, 2 * ct + ich : 2 * ct + ich + 1],
                    )
                    nc.vector.tensor_sub(
                        fb16_sb[:, ct, io], xp_sb[:, ct, io],
                        ff_sb[:, ct, io].bitcast(f32),
                    )
                # local K2 conv chunk
                k2l = l2pool.tile([D, IC], bf16, tag="k2l", name="k2l")
                ps = psC.tile([D, IC], f32, tag="c", name="k2l_ps")
                nc.tensor.matmul(
                    ps[:], wk2[:, 0, :], fb16_sb[:, 0, io],
                    start=True, stop=False,
                )
                nc.tensor.matmul(
                    ps[:], wk2[:, 1, :], fb16_sb[:, 1, io],
                    start=False, stop=True,
                )
                # PSUM->SBUF copies on the Scalar engine: the DVE is backed
                # up with epilogue STTs here and would delay the gather
                nc.scalar.add(k2l[:], ps[:], consts_sb[0:D, 9:10])
                # local V2^T conv tiles
                v2l = l2pool.tile([P, 4, C], bf16, tag="v2l", name="v2l")
                for tsub in range(4):
                    cs = slice(io.start + tsub * P, io.start + (tsub + 1) * P)
                    psv = psC.tile([P, C], f32, tag="c", name="v2l_ps")
                    nc.tensor.matmul(
                        psv[:], fb16_sb[:, 0, cs], wv2[:, 0, :],
                        start=True, stop=False,
                    )
                    nc.tensor.matmul(
                        psv[:], fb16_sb[:, 1, cs], wv2[:, 1, :],
                        start=False, stop=True,
                    )
                    nc.scalar.copy(v2l[:, tsub, :], psv[:])
                nc.sync.dma_start(out=ag_ins[ich][0:D, :], in_=k2l[:])
                nc.gpsimd.dma_start(out=ag_ins[ich][D:, :], in_=v2l[:])
                nc.gpsimd.collective_compute(
                    "AllGather",
                    OP.bypass,
                    replica_groups=groups,
                    ins=[ag_ins[ich][:].opt()],
                    outs=[ag_outs[ich][:].opt()],
                )
                # Σff² (scratch output, accumulator is the point)
                for ct in range(2):
                    sq = sqpool.tile([P, IC], f32, tag="sq", name="ffsq")
                    nc.vector.scalar_tensor_tensor(
                        sq[:], ff_sb[:, ct, io].bitcast(f32),
                        1.0, ff_sb[:, ct, io].bitcast(f32),
                        op0=OP.mult, op1=OP.mult,
                        accum_out=sacc_sb[:, 4 + 2 * ct + ich
                                          : 5 + 2 * ct + ich],
                    )
                conv_qk(wq2, 8, lambda k, js: ff_sb[:, k, js], io, q2_sb)

            epilogue1(0, at0)
            del at0

            at1 = AttnChunk(q1_sb, k1_sb, v1_sb, 1, list(range(NT)))
            at1.emit(NT)
            epilogue1(1, at1)
            del at1

            # assemble ff stats: stats cols 0-1 = S1f (ct0, ct1),
            # cols 4-5 = S2f -- S1/S2 grouped 4-wide for the var chain
            for c in range(4):
                dst = (0, 1, 4, 5)[c]
                nc.vector.tensor_add(
                    stats_sb[:, dst : dst + 1],
                    sacc_sb[:, 2 * c : 2 * c + 1],
                    sacc_sb[:, 2 * c + 1 : 2 * c + 2],
                )

            # ============== Layer 2: masked cross-attention ==============
            # redistribute the gathered K2/V2 conv outputs: 2 transposed
            # DMAs per phase instead of per-rank copies
            v2_sb = big.tile([P, NT, C], bf16, tag="v2", name="v2_sb")
            for h in range(2):
                eng = (nc.sync, nc.gpsimd)[h]
                eng.dma_start(
                    out=k2_sb[:].rearrange("d (r q) -> d r q", r=RSH)[
                        :, :, h * IC : (h + 1) * IC
                    ],
                    in_=ag_outs[h][:, 0:D, :].transpose([1, 0, 2]),
                )
                eng.dma_start(
                    out=v2_sb[:].rearrange("p (r t) c -> p r t c", r=RSH)[
                        :, :, 4 * h : 4 * h + 4, :
                    ],
                    in_=ag_outs[h][:, D:, :].rearrange(
                        "r (p j) (x c) -> p r (j x) c", j=2, x=2
                    ),
                )

            ph_tiles = [
                [t for r in range(RSH)
                 for t in range((r * R + h * IC) // P,
                                (r * R + h * IC) // P + 4)]
                for h in range(2)
            ]

            bt0 = AttnChunk(q2_sb, k2_sb, v2_sb, 0,
                            ph_tiles[0] + ph_tiles[1])
            bt0.emit(NT)

            def epilogue2(ich, at):
                """normalized sw_bg chunk; accumulate per-channel sum/sumsq
                into sacc_sb cols 8-15 via the DVE accumulator."""
                rr = at.rrep()
                for ct in range(2):
                    onb = onpool.tile([P, IC], f32, tag="rr", name="on2")
                    nc.vector.scalar_tensor_tensor(
                        onb[:], at.accs[ct][:], 1.0, rr[:],
                        op0=OP.mult, op1=OP.mult,
                        accum_out=sacc_sb[:, 8 + 2 * ct + ich
                                          : 9 + 2 * ct + ich],
                    )
                    sqb = sqpool.tile([P, IC], f32, tag="sq", name="sq2")
                    nc.vector.scalar_tensor_tensor(
                        sqb[:], onb[:], 1.0, onb[:],
                        op0=OP.mult, op1=OP.mult,
                        accum_out=sacc_sb[:, 12 + 2 * ct + ich
                                          : 13 + 2 * ct + ich],
                    )

            epilogue2(0, bt0)
            del bt0
            bt1 = AttnChunk(q2_sb, k2_sb, v2_sb, 1, list(range(NT)))
            bt1.emit(NT)
            # preload the Sqrt ACT table while the last AV matmuls drain so
            # the tail's real sqrt skips the 1.3us table switch
            sqwarm = misc.tile([P, 1], f32, tag="sqwarm", name="sqwarm")
            nc.scalar.activation(sqwarm[:], consts_sb[:, 0:1], AF.Sqrt)
            epilogue2(1, bt1)
            del bt1

            # assemble sw_bg stats: stats cols 2-3 = S1g, cols 6-7 = S2g
            for c in range(4):
                dst = (2, 3, 6, 7)[c]
                nc.vector.tensor_add(
                    stats_sb[:, dst : dst + 1],
                    sacc_sb[:, 8 + 2 * c : 9 + 2 * c],
                    sacc_sb[:, 9 + 2 * c : 10 + 2 * c],
                )

            # ============ stats AllGather + local sum + FMM ============
            sg_in = dram.tile([P, 8], f32, tag="sg_in", name="sg_in")
            sg_out = dram.tile([RSH, P, 8], f32, tag="sg_out", name="sg_out")
            nc.sync.dma_start(out=sg_in[:], in_=stats_sb[:])
            nc.gpsimd.collective_compute(
                "AllGather",
                OP.bypass,
                replica_groups=groups,
                ins=[sg_in[:].opt()],
                outs=[sg_out[:].opt()],
            )
            rst4 = misc.tile([P, RSH, 8], f32, tag="rst4", name="rst4")
            nc.sync.dma_start(
                out=rst4[:], in_=sg_out[:].transpose([1, 0, 2])
            )
            rst = misc.tile([P, 8], f32, tag="rst", name="rst")
            nc.vector.tensor_add(rst[:], rst4[:, 0, :], rst4[:, 1, :])
            rstb = misc.tile([P, 8], f32, tag="rstb", name="rstb")
            nc.vector.tensor_add(rstb[:], rst4[:, 2, :], rst4[:, 3, :])
            nc.vector.tensor_add(rst[:], rst[:], rstb[:])

            # var = (S2 - S1^2/N)/(N-1) + EPS, all four (f/g x ct) at once
            var4 = misc.tile([P, 4], f32, tag="var4", name="var4")
            ratio = misc.tile([P, 2], f32, tag="ratio", name="ratio")
            nc.vector.tensor_mul(var4[:], rst[:, 0:4], rst[:, 0:4])
            nc.vector.tensor_scalar(
                var4[:], var4[:], -1.0 / N, None, op0=OP.mult
            )
            nc.vector.tensor_add(var4[:], var4[:], rst[:, 4:8])
            nc.vector.tensor_scalar(
                var4[:], var4[:], 1.0 / (N - 1), EPS, op0=OP.mult, op1=OP.add
            )
            varfi = misc.tile([P, 2], f32, tag="varfi", name="varfi")
            nc.vector.reciprocal(varfi[:], var4[:, 0:2])
            nc.vector.tensor_mul(varfi[:], varfi[:], var4[:, 2:4])
            nc.scalar.activation(ratio[:], varfi[:], AF.Sqrt)
            # fold in gamma
            nc.vector.tensor_scalar_mul(ratio[:], ratio[:], consts_sb[:, 1:2])

            # out = x' + (gamma * std_bg/std_f) * ff, in 4 chunks with the
            # store DMAs dispatched on 4 different engine queues
            outq = (nc.sync, nc.scalar, nc.gpsimd, nc.sync)
            for i, (ct, ih) in enumerate(
                ((0, 0), (1, 0), (0, 1), (1, 1))
            ):
                io = slice(ih * IC, (ih + 1) * IC)
                fin = finpool.tile([P, IC], f32, tag="fin", name="fin")
                nc.vector.scalar_tensor_tensor(
                    fin[:], ff_sb[:, ct, io].bitcast(f32),
                    ratio[:, ct : ct + 1], xp_sb[:, ct, io],
                    op0=OP.mult, op1=OP.add,
                )
                outq[i].dma_start(
                    out=out_d[ct * P : (ct + 1) * P, io], in_=fin[:]
                )

    nc.compile()
    return nc


def _prep_inputs(x, mask, sa_wq, sa_bq, sa_wk, sa_bk, sa_wv, sa_bv, sa_gamma,
                 wq, bq, wk, bk, wv, bv, gamma):
    """Build the per-core input maps (host-side sharding + weight layout)."""
    x = np.ascontiguousarray(x, dtype=F32)
    mask = np.ascontiguousarray(mask, dtype=F32)

    import ml_dtypes

    BF16 = ml_dtypes.bfloat16
    def chalf(a):
        """[C, X] -> [128, 2*X] with the two channel halves interleaved
        per partition (so device DMAs are contiguous)."""
        X = a.shape[1]
        return np.ascontiguousarray(
            a.reshape(2, P, X).transpose(1, 0, 2).reshape(P, 2 * X)
        )

    # packed weights: f32r pack = wq2; bf16 pack = wq1|wk1|wk2|wv1|wv2
    wpackr = chalf(np.ascontiguousarray(wq.T, dtype=F32))
    wpackb = chalf(
        np.concatenate(
            [sa_wq.T, sa_wk.T, wk.T, sa_wv.T, wv.T], axis=1
        ).astype(BF16)
    )

    consts = np.zeros((P, 10), dtype=F32)
    consts[:, 0] = sa_gamma[0]
    consts[:, 1] = gamma[0]
    sgb = (sa_gamma[0] * sa_bv).astype(F32)
    consts[:, 2] = sgb[0:P]
    consts[:, 3] = sgb[P:C]
    consts[0:D, 6] = sa_bq
    consts[0:D, 7] = sa_bk
    consts[0:D, 8] = bq
    consts[0:D, 9] = bk

    in_maps = []
    for g in range(NCORES):
        b, r = g // RSH, g % RSH
        xb = np.ascontiguousarray(x[b].reshape(C, N))
        mb = np.ascontiguousarray(mask[b].reshape(1, N))
        xloc = xb[:, r * R : (r + 1) * R]
        # xf: [128, NCH, 2, CW] host layout -> per-chunk contiguous DMAs
        NCH, CW = 4, N // 4
        xfp = np.ascontiguousarray(
            xb.astype(BF16).reshape(2, P, NCH, CW)
            .transpose(1, 2, 0, 3).reshape(P, 2 * N)
        )
        in_maps.append({
            "xf": xfp,
            "xc": chalf(xloc),
            "xcb": chalf(xloc.astype(BF16)),
            "mcrow": np.ascontiguousarray(mb[:, r * R : (r + 1) * R]),
            "wpackr": wpackr, "wpackb": wpackb,
            "consts": consts,
        })
    return in_maps


def kernel(**inputs):
    from concourse import bass_utils

    if "nc" not in _CACHE:
        _CACHE["nc"] = _build_bass()
    nc = _CACHE["nc"]

    in_maps = _prep_inputs(**inputs)
    res = bass_utils.run_bass_kernel_spmd(
        nc, in_maps, core_ids=list(range(NCORES))
    )
    _CACHE["last_results"] = res

    out = np.empty((B, C, N), dtype=F32)
    for g in range(NCORES):
        b, r = g // RSH, g % RSH
        out[b, :, r * R : (r + 1) * R] = res.results[g]["outc"]
    return out.reshape(B, C, HH, WW)
